# revision 1
# baseline (speedup 1.0000x reference)
"""Multi-head cross-attention on 8 Trainium2 NeuronCores.

Sharding: data-parallel over batch (2) x tensor-parallel over heads (4 groups
of 4 heads). Core c handles batch c//4, head-group c%4 (a 256-wide slice of
the QKV projection space). Each core computes a partial output-projection
Y_partial = ctx_c @ Wo_c; a ReduceScatter(add) over each batch's 4 cores
leaves each core with a 512-row shard of the summed output, which the host
concatenates.

On-core dataflow (all matmuls in fp32r at full PE rate):
  - x is PE-transposed to d-major (in two d-halves to halve SBUF residency;
    projections accumulate the halves via an SBUF add). Q^T/K^T = W.T @ x^T
    come out j-major, V = x @ Wv comes out s-major -- exactly the operand
    layouts the attention matmuls need, so no other transposes occur.
  - scores are built k-major (S^T) two PSUM banks at a time, exp'd in one
    [128,1024] scalar-engine op (no max subtraction: scores ~ N(0,1)), and
    fed straight into the PV matmul. V carries 64 ones-columns so the softmax
    denominator lands in PSUM partitions 64..127 of the same matmul; a single
    PSUM-to-PSUM tensor divide normalizes while evicting to SBUF.
  - bq/bk are applied on-device (per-partition bias in j-major layout).
    bv/bo commute through softmax/out-projection exactly (softmax rows sum
    to 1), so the host adds bv @ Wo + bo to the final output.
"""

import numpy as np

B, SEQ, D, H, DH = 2, 2048, 1024, 16, 64
N_CORES = 8
GROUPS = 4            # head-groups per batch (cores per batch)
JG = D // GROUPS      # 256 projection dims per core
HPC = H // GROUPS     # 4 heads per core
P = 128

_cached = {}


def _build_program(seq=SEQ, use_f32r=True, with_collective=True,
                   exp_width=1024):
    import concourse.tile as tile
    from concourse import bacc, mybir
    from concourse.masks import make_identity

    F32 = mybir.dt.float32
    MMT = mybir.dt.float32r if use_f32r else mybir.dt.float32

    def mm(x):
        return x.bitcast(MMT)

    # producers of matmul operands must write rounded f32r (walrus birverifier)
    r = mm

    s_chunks = seq // P          # 16  (128-row chunks)
    sb_chunks = seq // 512       # 4   (512-wide blocks)
    sk_chunks = seq // 1024      # 2   (1024-wide attention blocks)
    d_chunks = D // P            # 8
    dh_chunks = d_chunks // 2    # 4   (per d-half)
    j_chunks = JG // P           # 2

    nc = bacc.Bacc("TRN2", target_bir_lowering=False, debug=False,
                   num_devices=N_CORES)

    x1b = nc.dram_tensor("x1b", [seq, D], F32, kind="ExternalInput")
    x2b = nc.dram_tensor("x2b", [seq, D], F32, kind="ExternalInput")
    wq = nc.dram_tensor("wq", [D, JG], F32, kind="ExternalInput")
    wk = nc.dram_tensor("wk", [D, JG], F32, kind="ExternalInput")
    wv = nc.dram_tensor("wv", [D, JG], F32, kind="ExternalInput")
    wo = nc.dram_tensor("wo", [JG, D], F32, kind="ExternalInput")
    bqr = nc.dram_tensor("bqr", [P, j_chunks], F32, kind="ExternalInput")
    bkr = nc.dram_tensor("bkr", [P, j_chunks], F32, kind="ExternalInput")
    y_out = nc.dram_tensor("y_out", [seq // GROUPS, D], F32,
                           kind="ExternalOutput")

    EXP = mybir.ActivationFunctionType.Exp
    DIV = mybir.AluOpType.divide

    with tile.TileContext(nc) as tc:
        with (
            tc.tile_pool(name="consts", bufs=1) as consts,
            tc.tile_pool(name="wqkv", bufs=3) as wqkv_pool,
            tc.tile_pool(name="wop", bufs=1) as wo_pool,
            tc.tile_pool(name="xload", bufs=3) as xload,
            tc.tile_pool(name="xt", bufs=2) as xt_pool,
            tc.tile_pool(name="acts", bufs=1) as acts,
            tc.tile_pool(name="ctp", bufs=2) as ct_pool,
            tc.tile_pool(name="epool", bufs=4) as epool,
            tc.tile_pool(name="small", bufs=2) as small,
            tc.tile_pool(name="ysb", bufs=4) as ysb,
            tc.tile_pool(name="psum_mm", bufs=2, space="PSUM") as psum_mm,
            tc.tile_pool(name="psum_s", bufs=(2 if exp_width == 1024 else 4), space="PSUM") as psum_s,
            tc.tile_pool(name="psum_u", bufs=2, space="PSUM") as psum_u,
            tc.tile_pool(name="dram", bufs=1, space="DRAM") as dram,
        ):
            ident = consts.tile([P, P], F32)
            make_identity(nc, ident)

            def load_weight_cast(wsb, w_dram, n_outer, width, pat):
                # DMA f32 chunks then cast into the f32r operand tile
                for o in range(n_outer):
                    st = ysb.tile([P, 1024], F32, tag="y",
                                  name=f"wst_{wsb.name}_{o}")
                    nc.sync.dma_start(
                        st[:, :width],
                        w_dram.rearrange(pat, p=P)[:, o, :])
                    nc.vector.tensor_copy(r(wsb[:, o, :]), st[:, :width])

            def load_slab(x_dram, sb):
                # two 1MB DMAs per slab; tile q-pair layout [P, 2, D]
                pairs = []
                for g in range(2):
                    xt_ = xload.tile([P, 2, D], F32, tag="xload")
                    nc.sync.dma_start(
                        xt_[:],
                        x_dram[(sb * 4 + 2 * g) * P:(sb * 4 + 2 * g + 2) * P,
                               :].rearrange("(q p) d -> p q d", p=P))
                    pairs.append(xt_)
                return [pairs[q // 2][:, q % 2] for q in range(4)]

            def transpose_slab(x_dram, sb, use_act=False, xts=None):
                # x rows [sb*512, (sb+1)*512) x full D -> xT [P, d_chunks, 512]
                # (d-major). In phase A (use_act) the idle 2-bank score slots
                # hold 8 batched PE transposes evicted by ONE [128,1024] copy,
                # alternating ACT/DVE; during attention (x1) fall back to
                # single-bank "mm" tiles so the score slots stay free.
                if xts is None:
                    xts = load_slab(x_dram, sb)
                xT = xt_pool.tile([P, d_chunks, 512], F32, tag="xT")
                if use_act:
                    for dg in range(d_chunks // 2):
                        pt = psum_s.tile([P, 1024], F32, tag="s",
                                         name=f"ptx_{x_dram.name}_{sb}_{dg}")
                        for i in range(2):
                            dc = 2 * dg + i
                            for q in range(4):
                                nc.tensor.transpose(
                                    pt[:, i * 512 + q * P:
                                       i * 512 + (q + 1) * P],
                                    xts[q][:, dc * P:(dc + 1) * P], ident[:])
                        out2 = xT[:, 2 * dg:2 * dg + 2, :]
                        if dg % 2 == 1:
                            nc.scalar.copy(r(out2), pt[:])
                        else:
                            nc.vector.tensor_copy(r(out2), pt[:])
                else:
                    for dc in range(d_chunks):
                        pt = psum_mm.tile([P, 512], F32, tag="mm")
                        for q in range(4):
                            nc.tensor.transpose(
                                pt[:, q * P:(q + 1) * P],
                                xts[q][:, dc * P:(dc + 1) * P], ident[:])
                        nc.vector.tensor_copy(r(xT[:, dc, :]), pt[:])
                return xT

            # x2 slab 0 loads go first so transposes start immediately;
            # weight DMAs stream in behind them
            x2tiles0 = load_slab(x2b, 0)

            # qkv weights rotate through 2 shared slots (k, v, then q)
            wk_sb = wqkv_pool.tile([P, d_chunks, JG], F32, tag="wqkv")
            wv_sb = wqkv_pool.tile([P, d_chunks, JG], F32, tag="wqkv")
            wo_sb = wo_pool.tile([P, j_chunks, D], F32, tag="wo")
            load_weight_cast(wk_sb, wk, d_chunks, JG, "(o p) j -> p o j")
            load_weight_cast(wv_sb, wv, d_chunks, JG, "(o p) j -> p o j")
            load_weight_cast(wo_sb, wo, j_chunks, D, "(o p) n -> p o n")
            bq_sb = consts.tile([P, j_chunks], F32, tag="bq")
            bk_sb = consts.tile([P, j_chunks], F32, tag="bk")
            nc.sync.dma_start(bq_sb[:], bqr[:])
            nc.sync.dma_start(bk_sb[:], bkr[:])


            kT = acts.tile([P, j_chunks, seq], F32, tag="kT")
            qT = acts.tile([P, j_chunks, seq], F32, tag="qT")
            # V'' per head-column-block: cols 0..63 V_h, 64..127 ones
            vpp = acts.tile([P, s_chunks, HPC * P], F32, tag="vpp")

            ones_f32 = consts.tile([P, DH], F32, tag="ones")
            nc.vector.memset(ones_f32[:], 1.0)
            for si in range(s_chunks):
                ones_view = vpp[:, si].rearrange("p (h q) -> p h q", q=P)[:, :, DH:P]
                nc.vector.tensor_copy(
                    r(ones_view),
                    ones_f32[:, None, :].to_broadcast([P, HPC, DH]))

            def project_jmajor(xT_s, w_sb, sb, out, bias, use_act=False,
                               on_s=False):
                # out[j, sb-slab] = w.T @ xT_s + bias. on_s borrows the
                # attention score PSUM banks (idle before the first exp) so
                # projections pipeline in parallel with the next slab's
                # transposes instead of contending for the 2 "mm" slots.
                ssl = slice(sb * 512, (sb + 1) * 512)
                for jc in range(j_chunks):
                    if on_s:
                        pk = psum_s.tile([P, 512], F32, tag="s",
                                         name=f"pk_{w_sb.name}_{sb}_{jc}")
                    else:
                        pk = psum_mm.tile([P, 512], F32, tag="mm")
                    for dc in range(d_chunks):
                        nc.tensor.matmul(
                            pk[:],
                            mm(w_sb[:, dc, jc * P:(jc + 1) * P]),
                            mm(xT_s[:, dc, :]),
                            start=(dc == 0), stop=(dc == d_chunks - 1))
                    if use_act:
                        nc.scalar.add(r(out[:, jc, ssl]), pk[:],
                                      bias[:, jc:jc + 1])
                    else:
                        nc.vector.tensor_scalar_add(
                            r(out[:, jc, ssl]), pk[:], bias[:, jc:jc + 1])

            def project_v(xT_s, sb):
                # V[s-slab, j] = x2_slab @ Wv into the vpp head blocks
                for q in range(4):
                    si = sb * 4 + q
                    pv = psum_u.tile([P, JG], F32, tag="u")
                    for dc in range(d_chunks):
                        nc.tensor.matmul(
                            pv[:],
                            mm(xT_s[:, dc, q * P:(q + 1) * P]),
                            mm(wv_sb[:, dc, :]),
                            start=(dc == 0), stop=(dc == d_chunks - 1))
                    vv = vpp[:, si].rearrange("p (h q) -> p h q", q=P)[:, :, 0:DH]
                    nc.vector.tensor_copy(
                        r(vv), pv[:].rearrange("p (h q) -> p h q", q=DH))

            ybounce = dram.tile([seq, D], F32, tag="yin")

            cts = {}
            pus_by = {}

            def emit_oproj(sc, cT):
                for s8 in range(8):
                  with nc.named_scope("oproj"):
                    si = sc * 8 + s8
                    yt = ysb.tile([P, D], F32, tag="y",
                                  name=f"yt_{sc}_{s8}")
                    last = sc == sk_chunks - 1
                    for nck in range(2):
                        if last and (s8 * 2 + nck) % 2 == 1:
                            py = psum_s.tile([P, 512], F32, tag="s",
                                             name=f"py_{sc}_{s8}_{nck}")
                        else:
                            py = psum_mm.tile([P, 512], F32, tag="mm",
                                              name=f"py_{sc}_{s8}_{nck}")
                        for jc in range(j_chunks):
                            nc.tensor.matmul(
                                py[:],
                                mm(cT[:, jc, s8 * P:(s8 + 1) * P]),
                                mm(wo_sb[:, jc, nck * 512:(nck + 1) * 512]),
                                start=(jc == 0), stop=(jc == j_chunks - 1))
                        if last:
                            nc.scalar.copy(
                                yt[:, nck * 512:(nck + 1) * 512], py[:])
                        else:
                            nc.vector.tensor_copy(
                                yt[:, nck * 512:(nck + 1) * 512], py[:])
                    nc.sync.dma_start(ybounce[si * P:(si + 1) * P, :], yt[:])

            def emit_pv(sc, h, kc, et):
                jc, po = h // 2, (h % 2) * DH
                if kc == 0:
                    pus_by[(sc, h)] = [
                        psum_u.tile([P, 512], F32, tag="u",
                                    name=f"pu_{sc}_{h}_{i}")
                        for i in range(2)]
                pus = pus_by[(sc, h)]
                for half in range(2):
                    fsl = slice(half * 512, (half + 1) * 512)
                    nc.tensor.matmul(
                        pus[half][:],
                        mm(vpp[:, kc, h * P:(h + 1) * P]),
                        mm(et[:, fsl]),
                        start=(kc == 0), stop=(kc == s_chunks - 1))
                if kc == s_chunks - 1:
                    cT = cts[sc]
                    for half in range(2):
                        fsl = slice(half * 512, (half + 1) * 512)
                        rt = small.tile([DH, 512], F32, tag="rt",
                                        name=f"rt_{sc}_{h}_{half}")
                        nc.vector.reciprocal(rt[:], pus[half][DH:P, :])
                        nc.vector.tensor_mul(
                            r(cT[po:po + DH, jc, fsl]),
                            pus[half][0:DH, :], rt[:])
                    del pus_by[(sc, h)]
                    if h == HPC - 1:
                        emit_oproj(sc, cT)

            pend = []

            def emit_attn_unit(sc, h, kc):
              with nc.named_scope("attn"):
                if (h, kc) == (0, 0):
                    cts[sc] = ct_pool.tile([P, j_chunks, 1024], F32,
                                           tag="cT", name=f"cT_{sc}")
                jc, po = h // 2, (h % 2) * DH
                ps = psum_s.tile([P, 1024], F32, tag="s",
                                 name=f"ps_{sc}_{h}_{kc}")
                for half in range(2):
                    hsl = slice(sc * 1024 + half * 512,
                                sc * 1024 + (half + 1) * 512)
                    nc.tensor.matmul(
                        ps[:, half * 512:(half + 1) * 512],
                        mm(kT[po:po + DH, jc, kc * P:(kc + 1) * P]),
                        mm(qT[po:po + DH, jc, hsl]),
                        start=True, stop=True)
                et = epool.tile([P, 1024], F32, tag="e",
                                name=f"et_{sc}_{h}_{kc}")
                nc.scalar.activation(r(et[:]), ps[:], EXP, scale=0.125)
                pend.append((sc, h, kc, et))
                if len(pend) > 3:
                    emit_pv(*pend.pop(0))

            # ---- x2 -> K^T, V'' (per 512-row slab) ----
            for sb in range(sb_chunks):
                with nc.named_scope("x2t"):
                    x2T_s = transpose_slab(x2b, sb, use_act=True,
                                           xts=(x2tiles0 if sb == 0 else None))
                with nc.named_scope("kproj"):
                    project_jmajor(x2T_s, wk_sb, sb, kT, bk_sb, use_act=True)
                with nc.named_scope("vproj"):
                    project_v(x2T_s, sb)

            # ---- x1 -> Q^T (per slab; overlaps with attention below) ----
            wq_sb = wqkv_pool.tile([P, d_chunks, JG], F32, tag="wqkv")
            load_weight_cast(wq_sb, wq, d_chunks, JG, "(o p) j -> p o j")
            for sb in range(sb_chunks):
                with nc.named_scope("x1t"):
                    x1T_s = transpose_slab(x1b, sb)
                with nc.named_scope("qproj"):
                    project_jmajor(x1T_s, wq_sb, sb, qT, bq_sb, on_s=(sb < 2))

            # ---- attention units (flat, PV lagging exp by 2) ----
            for sc in range(sk_chunks):
                for h in range(HPC):
                    for kc in range(s_chunks):
                        emit_attn_unit(sc, h, kc)
            with nc.named_scope("attn"):
                for args in pend:
                    emit_pv(*args)

            # ---- sum partials across the 4 cores of this batch ----
            # Two half-sized ReduceScatters: the first depends only on the
            # first 1024 rows (written when attention chunk 0's out-projection
            # lands), so it overlaps chunk 1's attention instead of
            # serializing after all compute.
            if with_collective:
                half = seq // 2                 # 1024 rows per collective
                qr = seq // GROUPS // 2         # 256 rows per rank per half
                for ci in range(2):
                    ysc = dram.tile([qr, D], F32, tag="yout",
                                    name=f"ysc_{ci}")
                    nc.gpsimd.collective_compute(
                        "ReduceScatter",
                        mybir.AluOpType.add,
                        replica_groups=[[0, 1, 2, 3], [4, 5, 6, 7]],
                        ins=[ybounce[ci * half:(ci + 1) * half, :].opt()],
                        outs=[ysc[:].opt()],
                    )
                    nc.sync.dma_start(y_out[ci * qr:(ci + 1) * qr, :], ysc[:])
            else:
                nc.sync.dma_start(y_out[:], ybounce[:seq // GROUPS, :])

    nc.compile()
    return nc


def _get_program(seq=SEQ, use_f32r=True):
    key = (seq, use_f32r)
    if key not in _cached:
        _cached[key] = _build_program(seq, use_f32r)
    return _cached[key]


def make_in_maps(x1, x2, Wq, bq, Wk, bk, Wv, bv, Wo, bo):
    """Per-core input dicts for the SPMD program."""
    in_maps = []
    for c in range(N_CORES):
        b, g = c // GROUPS, c % GROUPS
        js = slice(g * JG, (g + 1) * JG)
        in_maps.append({
            "x1b": np.ascontiguousarray(x1[b]),
            "x2b": np.ascontiguousarray(x2[b]),
            "wq": np.ascontiguousarray(Wq[:, js]),
            "wk": np.ascontiguousarray(Wk[:, js]),
            "wv": np.ascontiguousarray(Wv[:, js]),
            "wo": np.ascontiguousarray(Wo[js, :]),
            "bqr": np.ascontiguousarray(bq[js].reshape(2, P).T),
            "bkr": np.ascontiguousarray(bk[js].reshape(2, P).T),
        })
    return in_maps


def assemble(results, Wv_bias_fix):
    """results: list of per-core {'y_out': [seq//GROUPS, D]}.

    y_out rows [0:q) = rank's quarter of input rows [0:seq/2);
    rows [q:2q) = rank's quarter of input rows [seq/2:seq)."""
    seq = results[0]["y_out"].shape[0] * GROUPS
    q = seq // GROUPS // 2
    Y = np.empty((B, seq, D), np.float32)
    for c in range(N_CORES):
        b, rr = c // GROUPS, c % GROUPS
        yo = results[c]["y_out"]
        Y[b, rr * q:(rr + 1) * q, :] = yo[:q]
        Y[b, seq // 2 + rr * q:seq // 2 + (rr + 1) * q, :] = yo[q:]
    Y += Wv_bias_fix
    return Y


def kernel(x1, x2, Wq, bq, Wk, bk, Wv, bv, Wo, bo):
    from concourse.bass_utils import run_bass_kernel_spmd

    x1 = np.asarray(x1, np.float32)
    x2 = np.asarray(x2, np.float32)
    Wq, bq = np.asarray(Wq, np.float32), np.asarray(bq, np.float32)
    Wk, bk = np.asarray(Wk, np.float32), np.asarray(bk, np.float32)
    Wv, bv = np.asarray(Wv, np.float32), np.asarray(bv, np.float32)
    Wo, bo = np.asarray(Wo, np.float32), np.asarray(bo, np.float32)

    nc = _get_program(SEQ)
    in_maps = make_in_maps(x1, x2, Wq, bq, Wk, bk, Wv, bv, Wo, bo)
    res = run_bass_kernel_spmd(nc, in_maps, core_ids=list(range(N_CORES)))
    fix = (bv @ Wo + bo).astype(np.float32)
    return assemble(res.results, fix)



# revision 15
# speedup vs baseline: 1.2374x; 1.2374x over previous
"""Multi-head cross-attention on 8 Trainium2 NeuronCores.

Sharding: data-parallel over batch (2) x tensor-parallel over heads (4 groups
of 4 heads). Core c handles batch c//4, head-group c%4 (a 256-wide slice of
the QKV projection space). Each core computes a partial output-projection
Y_partial = ctx_c @ Wo_c; a ReduceScatter(add) over each batch's 4 cores
leaves each core with a 512-row shard of the summed output, which the host
concatenates.

On-core dataflow (all matmul operands bf16, f32 PSUM accumulation; the host
pre-casts x and W to bf16):
  - x^T (d-major) comes straight from DMA xbar transposes (dma_start_transpose
    on 512-row slabs) -- no PE transposes, no eviction copies.
  - Q^T/K^T = W.T @ x^T come out j-major, V = x^T-stationary @ Wv comes out
    s-major -- exactly the operand layouts the attention matmuls need.
  - scores are built k-major (S^T) two PSUM banks at a time and exp'd in one
    [128,1024] scalar-engine op (no max subtraction: scores ~ N(0,1)) with
    bf16 output. The scalar engine's exp throughput (~1.04us per [128,1024])
    is the steady-state bottleneck, so emission interleaves projection slabs
    into the first attention block to start exp'ing as early as possible.
  - PV streams [V_h | ones] (65 cols) against stationary 128-wide exp chunks:
    half the streamed rows of the V-stationary arrangement, with the softmax
    denominator accumulating in column 64. ctx is normalized during PSUM
    eviction (per-partition scalar multiply), PE-transposed to j-major in
    [64,512] groups, and fed to the out-projection. Out-projections are
    interleaved into the next attention block (sc0) or split into per-jc
    partial sums (sc1) to shorten the tail.
  - bq/bk are applied on-device (per-partition bias in j-major layout).
    bv/bo commute through softmax/out-projection exactly (softmax rows sum
    to 1), so the host adds bv @ Wo + bo to the final output.
"""

import numpy as np
import ml_dtypes

B, SEQ, D, H, DH = 2, 2048, 1024, 16, 64
N_CORES = 8
GROUPS = 4            # head-groups per batch (cores per batch)
JG = D // GROUPS      # 256 projection dims per core
HPC = H // GROUPS     # 4 heads per core
P = 128
QC = 8                # 128-wide q chunks per 1024-wide attention block
VW = DH + 1           # V block width: 64 V cols + 1 ones col (denominator)

_cached = {}


def _build_program(seq=SEQ, with_collective=True):
    import concourse.tile as tile
    from concourse import bacc, mybir
    from concourse.masks import make_identity

    F32 = mybir.dt.float32
    BF16 = mybir.dt.bfloat16

    s_chunks = seq // P          # 16  (128-row chunks)
    sb_chunks = seq // 512       # 4   (512-wide slabs)
    sk_chunks = seq // 1024      # 2   (1024-wide attention blocks)
    d_chunks = D // P            # 8
    j_chunks = JG // P           # 2

    nc = bacc.Bacc("TRN2", target_bir_lowering=False, debug=False,
                   num_devices=N_CORES)

    # host-pretransposed activations: [D, seq] d-major
    x1t = nc.dram_tensor("x1t", [D, seq], BF16, kind="ExternalInput")
    x2t = nc.dram_tensor("x2t", [D, seq], BF16, kind="ExternalInput")
    wq = nc.dram_tensor("wq", [D, JG], BF16, kind="ExternalInput")
    wk = nc.dram_tensor("wk", [D, JG], BF16, kind="ExternalInput")
    wv = nc.dram_tensor("wv", [D, JG], BF16, kind="ExternalInput")
    wo = nc.dram_tensor("wo", [JG, D], BF16, kind="ExternalInput")
    bqr = nc.dram_tensor("bqr", [P, j_chunks], F32, kind="ExternalInput")
    bkr = nc.dram_tensor("bkr", [P, j_chunks], F32, kind="ExternalInput")
    y_out = nc.dram_tensor("y_out", [seq // GROUPS, D], BF16,
                           kind="ExternalOutput")

    EXP = mybir.ActivationFunctionType.Exp
    ADD = mybir.AluOpType.add

    with tile.TileContext(nc) as tc:
        with (
            tc.tile_pool(name="consts", bufs=1) as consts,
            tc.tile_pool(name="wts", bufs=1) as wts,
            tc.tile_pool(name="xt2", bufs=4) as xt2_pool,
            tc.tile_pool(name="xt1", bufs=4) as xt1_pool,
            tc.tile_pool(name="acts", bufs=1) as acts,
            tc.tile_pool(name="epool", bufs=4) as epool,
            tc.tile_pool(name="rpool", bufs=2) as rpool,
            tc.tile_pool(name="cnp", bufs=4) as cnp,
            tc.tile_pool(name="ctp", bufs=2) as ctp,
            tc.tile_pool(name="ypart", bufs=1) as ypart,
            tc.tile_pool(name="ysb", bufs=4) as ysb,
            tc.tile_pool(name="psum_s", bufs=2, space="PSUM") as psum_s,
            tc.tile_pool(name="psum_pv", bufs=1, space="PSUM") as psum_pv,
            tc.tile_pool(name="psum_mm", bufs=2, space="PSUM") as psum_mm,
            tc.tile_pool(name="dram", bufs=1, space="DRAM") as dram,
        ):
            ident = consts.tile([P, P], BF16)
            make_identity(nc, ident)

            # weights stream in bf16, already in operand layout
            wk_sb = wts.tile([P, d_chunks, JG], BF16, tag="wk")
            wv_sb = wts.tile([P, d_chunks, JG], BF16, tag="wv")
            wq_sb = wts.tile([P, d_chunks, JG], BF16, tag="wq")
            wo_sb = wts.tile([P, j_chunks, D], BF16, tag="wo")
            bq_sb = consts.tile([P, j_chunks], F32, tag="bq")
            bk_sb = consts.tile([P, j_chunks], F32, tag="bk")

            kT = acts.tile([P, j_chunks, seq], BF16, tag="kT")
            qT = acts.tile([P, j_chunks, seq], BF16, tag="qT")
            # V in s-major head blocks [V_h | ones]: vpp[:, si, h*VW:(h+1)*VW]
            vpp = acts.tile([P, s_chunks, HPC * VW], BF16, tag="vpp")
            ones_view = vpp.rearrange("p s (h c) -> p s h c", c=VW)[:, :, :, DH]
            nc.vector.memset(ones_view, 1.0)

            # DMA emission order = transfer order: feed the first attention
            # block as early as possible.
            x2tiles, x1tiles = [], []

            def emit_xt(pool, tiles, xt_dram, sb, tag):
                t = pool.tile([P, d_chunks, 512], BF16, tag=tag,
                              name=f"{tag}_{sb}")
                nc.sync.dma_start(
                    t[:],
                    xt_dram.rearrange("(o p) s -> p o s", p=P)[
                        :, :, sb * 512:(sb + 1) * 512])
                tiles.append(t)

            nc.sync.dma_start(wk_sb[:], wk.rearrange("(o p) j -> p o j", p=P))
            nc.sync.dma_start(bk_sb[:], bkr[:])
            emit_xt(xt2_pool, x2tiles, x2t, 0, "x2T")
            nc.sync.dma_start(wq_sb[:], wq.rearrange("(o p) j -> p o j", p=P))
            nc.sync.dma_start(bq_sb[:], bqr[:])
            emit_xt(xt1_pool, x1tiles, x1t, 0, "x1T")
            emit_xt(xt1_pool, x1tiles, x1t, 1, "x1T")
            nc.sync.dma_start(wv_sb[:], wv.rearrange("(o p) j -> p o j", p=P))
            emit_xt(xt2_pool, x2tiles, x2t, 1, "x2T")
            emit_xt(xt2_pool, x2tiles, x2t, 2, "x2T")
            emit_xt(xt2_pool, x2tiles, x2t, 3, "x2T")
            emit_xt(xt1_pool, x1tiles, x1t, 2, "x1T")
            emit_xt(xt1_pool, x1tiles, x1t, 3, "x1T")
            nc.sync.dma_start(wo_sb[:], wo.rearrange("(o p) n -> p o n", p=P))

            def project_jmajor(xT_s, w_sb, sb, out, bias, pool):
                # out[j, sb-slab] = w.T @ xT_s + bias (k/q projections)
                ssl = slice(sb * 512, (sb + 1) * 512)
                for jc in range(j_chunks):
                    pk = pool.tile([P, 512], F32, tag=pool.name[5:],
                                   name=f"pk_{w_sb.name}_{sb}_{jc}")
                    for dc in range(d_chunks):
                        nc.tensor.matmul(
                            pk[:],
                            w_sb[:, dc, jc * P:(jc + 1) * P],
                            xT_s[:, dc, :],
                            start=(dc == 0), stop=(dc == d_chunks - 1))
                    nc.vector.tensor_scalar_add(
                        out[:, jc, ssl], pk[:], bias[:, jc:jc + 1])

            def project_v(xT_s, sb):
                # V[s-slab, j] = x2_slab @ Wv into the vpp head blocks
                for q in range(4):
                    si = sb * 4 + q
                    pv = psum_s.tile([P, JG], F32, tag="s",
                                     name=f"pvv_{sb}_{q}")
                    for dc in range(d_chunks):
                        nc.tensor.matmul(
                            pv[:],
                            xT_s[:, dc, q * P:(q + 1) * P],
                            wv_sb[:, dc, :],
                            start=(dc == 0), stop=(dc == d_chunks - 1))
                    vv = vpp[:, si].rearrange("p (h c) -> p h c", c=VW)
                    nc.vector.tensor_copy(
                        vv[:, :, 0:DH],
                        pv[:].rearrange("p (h c) -> p h c", c=DH))

            ybounce = dram.tile([seq, D], BF16, tag="yin")
            cts = {}
            pvs = {}
            pend = []

            def emit_pv(sc, h, kc, et):
              with nc.named_scope("attn"):
                first, fin = kc == 0, kc == s_chunks - 1
                for qc in range(QC):
                    # start/stop bracket the whole PSUM bank (2KB zero
                    # region): only the first sub-region starts it and only
                    # the last one stops it
                    nc.tensor.matmul(
                        pvs[(sc, h)][qc // 4][:, qc % 4, :],
                        et[:, qc * P:(qc + 1) * P],
                        vpp[:, kc, h * VW:(h + 1) * VW],
                        start=first and qc % 4 == 0,
                        stop=fin and qc % 4 == 3)
                if fin:
                    emit_attn_tail(sc, h)

            def emit_attn_unit(sc, h, kc):
              with nc.named_scope("attn"):
                jc, po = h // 2, (h % 2) * DH
                if (h, kc) == (0, 0):
                    cts[sc] = ctp.tile([P, j_chunks, QC * P], BF16, tag="ct",
                                       name=f"ct_{sc}")
                if kc == 0:
                    pvs[(sc, h)] = [
                        psum_pv.tile([P, 4, VW], F32, tag=f"pv{g}",
                                     name=f"pvb_{sc}_{h}_{g}")
                        for g in range(2)]
                ps = psum_s.tile([P, 1024], F32, tag="s",
                                 name=f"ps_{sc}_{h}_{kc}")
                for half in range(2):
                    hsl = slice(sc * 1024 + half * 512,
                                sc * 1024 + (half + 1) * 512)
                    nc.tensor.matmul(
                        ps[:, half * 512:(half + 1) * 512],
                        kT[po:po + DH, jc, kc * P:(kc + 1) * P],
                        qT[po:po + DH, jc, hsl],
                        start=True, stop=True)
                et = epool.tile([P, 1024], BF16, tag="e",
                                name=f"et_{sc}_{h}_{kc}")
                nc.scalar.activation(et[:], ps[:], EXP, scale=0.125)
                pend.append((sc, h, kc, et))
                # PV lags exp by 2 units so PE's in-order queue never parks
                # on an unfinished exp
                if len(pend) > 2:
                    emit_pv(*pend.pop(0))

            def emit_attn_tail(sc, h):
                # normalize ctx during eviction, then transpose to j-major
                jc, po = h // 2, (h % 2) * DH
                pvb = pvs.pop((sc, h))
                ct = cts[sc]
                with nc.named_scope("norm"):
                    rc = rpool.tile([P, 2, 4], F32, tag="rc",
                                    name=f"rc_{sc}_{h}")
                    for g in range(2):
                        nc.vector.reciprocal(rc[:, g], pvb[g][:, :, DH])
                    cn = cnp.tile([P, QC, DH], BF16, tag="cn",
                                  name=f"cn_{sc}_{h}")
                    for qc in range(QC):
                        nc.vector.tensor_scalar_mul(
                            cn[:, qc, :], pvb[qc // 4][:, qc % 4, 0:DH],
                            rc[:, qc // 4, qc % 4:qc % 4 + 1])
                with nc.named_scope("ctT"):
                    for g in range(2):
                        pt = psum_mm.tile([P, 4, P], BF16, tag="mm",
                                          name=f"pt_{sc}_{h}_{g}")
                        for i in range(4):
                            # one accumulation group per PSUM bank: first
                            # transpose starts it, last one stops it
                            nc.tensor.matmul(
                                pt[0:DH, i, :], cn[:, g * 4 + i, :],
                                ident[:], is_transpose=True,
                                start=(i == 0), stop=(i == 3))
                        out = ct[po:po + DH, jc,
                                 g * 512:(g + 1) * 512].rearrange(
                                     "p (a b) -> p a b", a=4)
                        nc.vector.tensor_copy(out, pt[0:DH])

            def emit_qproj_late(sb):
                # Q slabs 2-3 woven into the sc0 attention stream (mm banks)
                project_jmajor(x1tiles[sb], wq_sb, sb, qT, bq_sb, psum_mm)

            yp_sb = ypart.tile([P, QC, D], BF16, tag="yp")

            def emit_oproj(sc, s8_list, rows, mode, pool):
                # out-projection pass over 128-row chunks of Y contracting
                # the `rows` slice of ct/wo. mode: "full" (single pass),
                # "park" (first pass -> yp_sb), "acc" (add into yp_sb),
                # "fin" (last pass: re-add yp_sb in PSUM via an identity
                # matmul, evict split over ACT/DVE -- this is the kernel
                # tail, keep it short).
                ct = cts[sc]
                r0, r1 = rows
                jcs = (range(j_chunks) if (r0, r1) == (0, JG) else
                       [r0 // P])
                p0, p1 = r0 % P, (r1 - 1) % P + 1
                for s8 in s8_list:
                  with nc.named_scope("oproj"):
                    if mode in ("full", "fin"):
                        yt = ysb.tile([P, D], BF16, tag="y",
                                      name=f"yt_{sc}_{s8}")
                    for nck in range(2):
                        py = pool.tile([P, 512], F32, tag=pool.name[5:],
                                       name=f"py_{sc}_{s8}_{nck}_{r0}")
                        for i, jc in enumerate(jcs):
                            nc.tensor.matmul(
                                py[:],
                                ct[p0:p1, jc, s8 * P:(s8 + 1) * P],
                                wo_sb[p0:p1, jc, nck * 512:(nck + 1) * 512],
                                start=(i == 0),
                                stop=(i == len(jcs) - 1 and mode != "fin"))
                        osl = slice(nck * 512, (nck + 1) * 512)
                        if mode == "park":
                            nc.vector.tensor_copy(yp_sb[:, s8, osl], py[:])
                        elif mode == "acc":
                            nc.vector.tensor_tensor(
                                yp_sb[:, s8, osl], py[:], yp_sb[:, s8, osl],
                                ADD)
                        elif mode == "fin":
                            nc.tensor.matmul(
                                py[:], ident[:],
                                yp_sb[:, s8, osl],
                                start=False, stop=True)
                            if (s8 + nck) % 2:
                                nc.vector.tensor_copy(yt[:, osl], py[:])
                            else:
                                nc.scalar.copy(yt[:, osl], py[:])
                        else:
                            nc.vector.tensor_copy(yt[:, osl], py[:])
                    if mode in ("full", "fin"):
                        nc.sync.dma_start(
                            ybounce[(sc * QC + s8) * P:
                                    (sc * QC + s8 + 1) * P, :],
                            yt[:])

            # ---- phase 1: projections woven into attention block sc0 ----
            # K0/Q0/Q1 first (the first score's inputs), V0 next (first PV,
            # pend-lagged by 2 units)
            with nc.named_scope("proj0"):
                project_jmajor(x2tiles[0], wk_sb, 0, kT, bk_sb, psum_s)
                project_jmajor(x1tiles[0], wq_sb, 0, qT, bq_sb, psum_s)
                project_jmajor(x1tiles[1], wq_sb, 1, qT, bq_sb, psum_s)
                project_v(x2tiles[0], 0)
            for sb in range(1, sb_chunks):
                for kc in range((sb - 1) * 4, sb * 4):
                    emit_attn_unit(0, 0, kc)
                with nc.named_scope(f"proj{sb}"):
                    project_jmajor(x2tiles[sb], wk_sb, sb, kT, bk_sb, psum_s)
                    project_v(x2tiles[sb], sb)
            for kc in range(12, s_chunks):
                emit_attn_unit(0, 0, kc)
            emit_qproj_late(2)
            for kc in range(s_chunks):
                emit_attn_unit(0, 1, kc)
            emit_qproj_late(3)
            for h in (2, 3):
                for kc in range(s_chunks):
                    emit_attn_unit(0, h, kc)

            # ---- phase 2: attention sc1; oproj(sc0) + partial oproj(sc1)
            # woven in so only the last per-head pass lands in the tail ----
            for h in range(HPC):
                for kc in range(s_chunks):
                    emit_attn_unit(1, h, kc)
                if h == 0:
                    emit_oproj(0, list(range(QC)), (0, JG), "full", psum_mm)
                elif h == 2:
                    # ct1 jc0 (heads 0-1) done: park partial output sums
                    emit_oproj(1, list(range(QC)), (0, P), "park", psum_mm)
            # ct1 rows 128:192 (head 2, already transposed) accumulate while
            # head 3's last exps run; rows 192:256 finish on the idle score
            # banks after the pend drain
            emit_oproj(1, list(range(QC)), (P, P + DH), "acc", psum_mm)
            for args in pend:
                emit_pv(*args)
            pend.clear()
            emit_oproj(1, list(range(QC)), (P + DH, JG), "fin", psum_s)

            # ---- sum partials across the 4 cores of this batch ----
            if with_collective:
                half = seq // 2                 # 1024 rows per collective
                qr = seq // GROUPS // 2         # 256 rows per rank per half
                for ci in range(2):
                    ysc = dram.tile([qr, D], BF16, tag="yout",
                                    name=f"ysc_{ci}")
                    nc.gpsimd.collective_compute(
                        "ReduceScatter",
                        mybir.AluOpType.add,
                        replica_groups=[[0, 1, 2, 3], [4, 5, 6, 7]],
                        ins=[ybounce[ci * half:(ci + 1) * half, :].opt()],
                        outs=[ysc[:].opt()],
                    )
                    nc.sync.dma_start(y_out[ci * qr:(ci + 1) * qr, :], ysc[:])
            else:
                nc.sync.dma_start(y_out[:], ybounce[:seq // GROUPS, :])

    nc.compile()
    return nc


def _get_program(seq=SEQ):
    if seq not in _cached:
        _cached[seq] = _build_program(seq)
    return _cached[seq]


def make_in_maps(x1, x2, Wq, bq, Wk, bk, Wv, bv, Wo, bo):
    """Per-core input dicts for the SPMD program."""
    bf16 = ml_dtypes.bfloat16
    in_maps = []
    for c in range(N_CORES):
        b, g = c // GROUPS, c % GROUPS
        js = slice(g * JG, (g + 1) * JG)
        in_maps.append({
            "x1t": np.ascontiguousarray(x1[b].T.astype(bf16)),
            "x2t": np.ascontiguousarray(x2[b].T.astype(bf16)),
            "wq": np.ascontiguousarray(Wq[:, js].astype(bf16)),
            "wk": np.ascontiguousarray(Wk[:, js].astype(bf16)),
            "wv": np.ascontiguousarray(Wv[:, js].astype(bf16)),
            "wo": np.ascontiguousarray(Wo[js, :].astype(bf16)),
            "bqr": np.ascontiguousarray(bq[js].reshape(2, P).T),
            "bkr": np.ascontiguousarray(bk[js].reshape(2, P).T),
        })
    return in_maps


def assemble(results, Wv_bias_fix):
    """results: list of per-core {'y_out': [seq//GROUPS, D]}.

    y_out rows [0:q) = rank's quarter of input rows [0:seq/2);
    rows [q:2q) = rank's quarter of input rows [seq/2:seq)."""
    seq = results[0]["y_out"].shape[0] * GROUPS
    q = seq // GROUPS // 2
    Y = np.empty((B, seq, D), np.float32)
    for c in range(N_CORES):
        b, rr = c // GROUPS, c % GROUPS
        yo = np.asarray(results[c]["y_out"]).astype(np.float32)
        Y[b, rr * q:(rr + 1) * q, :] = yo[:q]
        Y[b, seq // 2 + rr * q:seq // 2 + (rr + 1) * q, :] = yo[q:]
    Y += Wv_bias_fix
    return Y


def kernel(x1, x2, Wq, bq, Wk, bk, Wv, bv, Wo, bo):
    from concourse.bass_utils import run_bass_kernel_spmd

    x1 = np.asarray(x1, np.float32)
    x2 = np.asarray(x2, np.float32)
    Wq, bq = np.asarray(Wq, np.float32), np.asarray(bq, np.float32)
    Wk, bk = np.asarray(Wk, np.float32), np.asarray(bk, np.float32)
    Wv, bv = np.asarray(Wv, np.float32), np.asarray(bv, np.float32)
    Wo, bo = np.asarray(Wo, np.float32), np.asarray(bo, np.float32)

    nc = _get_program(SEQ)
    in_maps = make_in_maps(x1, x2, Wq, bq, Wk, bk, Wv, bv, Wo, bo)
    res = run_bass_kernel_spmd(nc, in_maps, core_ids=list(range(N_CORES)))
    fix = (bv @ Wo + bo).astype(np.float32)
    return assemble(res.results, fix)


# revision 39
# speedup vs baseline: 1.3419x; 1.0845x over previous
"""Multi-head cross-attention on 8 Trainium2 NeuronCores.

Sharding: data-parallel over batch (2) x tensor-parallel over heads (4 groups
of 4 heads). Core c handles batch c//4, head-group c%4 (a 256-wide slice of
the QKV projection space). Each core computes a partial output-projection
Y_partial = ctx_c @ Wo_c; a ReduceScatter(add) over each batch's 4 cores
leaves each core with a 512-row shard of the summed output, which the host
concatenates.

On-core dataflow (all matmul operands bf16, f32 PSUM accumulation; the host
pre-casts x and W to bf16):
  - x^T (d-major) comes straight from DMA xbar transposes (dma_start_transpose
    on 512-row slabs) -- no PE transposes, no eviction copies.
  - Q^T/K^T = W.T @ x^T come out j-major, V = x^T-stationary @ Wv comes out
    s-major -- exactly the operand layouts the attention matmuls need.
  - scores are built k-major (S^T) two PSUM banks at a time and exp'd in one
    [128,1024] scalar-engine op (no max subtraction: scores ~ N(0,1)) with
    bf16 output. The scalar engine's exp throughput (~1.04us per [128,1024])
    is the steady-state bottleneck, so emission interleaves projection slabs
    into the first attention block to start exp'ing as early as possible.
  - PV streams [V_h | ones] (65 cols) against stationary 128-wide exp chunks:
    half the streamed rows of the V-stationary arrangement, with the softmax
    denominator accumulating in column 64. ctx is normalized during PSUM
    eviction (per-partition scalar multiply), PE-transposed to j-major in
    [64,512] groups, and fed to the out-projection. Out-projections are
    interleaved into the next attention block (sc0) or split into per-jc
    partial sums (sc1) to shorten the tail.
  - bq/bk are applied on-device (per-partition bias in j-major layout).
    bv/bo commute through softmax/out-projection exactly (softmax rows sum
    to 1), so the host adds bv @ Wo + bo to the final output.
"""

import numpy as np
import ml_dtypes

B, SEQ, D, H, DH = 2, 2048, 1024, 16, 64
N_CORES = 8
GROUPS = 4            # head-groups per batch (cores per batch)
JG = D // GROUPS      # 256 projection dims per core
HPC = H // GROUPS     # 4 heads per core
P = 128
QC = 8                # 128-wide q chunks per 1024-wide attention block
VW = DH + 1           # V block width: 64 V cols + 1 ones col (denominator)

_cached = {}


def _build_program(seq=SEQ, with_collective=True):
    import concourse.tile as tile
    from concourse import bacc, mybir
    from concourse.masks import make_identity

    F32 = mybir.dt.float32
    BF16 = mybir.dt.bfloat16

    s_chunks = seq // P          # 16  (128-row chunks)
    sb_chunks = seq // 512       # 4   (512-wide slabs)
    sk_chunks = seq // 1024      # 2   (1024-wide attention blocks)
    d_chunks = D // P            # 8
    j_chunks = JG // P           # 2

    nc = bacc.Bacc("TRN2", target_bir_lowering=False, debug=False,
                   num_devices=N_CORES)

    # host-pretransposed activations: [D, seq] d-major
    x1t = nc.dram_tensor("x1t", [D, seq], BF16, kind="ExternalInput")
    x2t = nc.dram_tensor("x2t", [D, seq], BF16, kind="ExternalInput")
    wq = nc.dram_tensor("wq", [D, JG], BF16, kind="ExternalInput")
    wk = nc.dram_tensor("wk", [D, JG], BF16, kind="ExternalInput")
    wv = nc.dram_tensor("wv", [D, JG], BF16, kind="ExternalInput")
    wo = nc.dram_tensor("wo", [JG, D], BF16, kind="ExternalInput")
    bqr = nc.dram_tensor("bqr", [P, j_chunks], F32, kind="ExternalInput")
    bkr = nc.dram_tensor("bkr", [P, j_chunks], F32, kind="ExternalInput")
    y_out = nc.dram_tensor("y_out", [seq // GROUPS, D], BF16,
                           kind="ExternalOutput")

    EXP = mybir.ActivationFunctionType.Exp
    ADD = mybir.AluOpType.add

    with tile.TileContext(nc) as tc:
        with (
            tc.tile_pool(name="consts", bufs=1) as consts,
            tc.tile_pool(name="wts", bufs=1) as wts,
            tc.tile_pool(name="xt2", bufs=4) as xt2_pool,
            tc.tile_pool(name="xt1", bufs=4) as xt1_pool,
            tc.tile_pool(name="acts", bufs=1) as acts,
            tc.tile_pool(name="epool", bufs=24) as epool,
            tc.tile_pool(name="rpool", bufs=2) as rpool,
            tc.tile_pool(name="cnp", bufs=4) as cnp,
            tc.tile_pool(name="ctp", bufs=2) as ctp,
            tc.tile_pool(name="ypart", bufs=1) as ypart,
            tc.tile_pool(name="ysb", bufs=4) as ysb,
            tc.tile_pool(name="psum_s", bufs=2, space="PSUM") as psum_s,
            tc.tile_pool(name="psum_pv", bufs=1, space="PSUM") as psum_pv,
            tc.tile_pool(name="psum_mm", bufs=2, space="PSUM") as psum_mm,
            tc.tile_pool(name="dram", bufs=1, space="DRAM") as dram,
        ):
            ident = consts.tile([P, P], BF16)
            make_identity(nc, ident)

            # weights stream in bf16, already in operand layout
            wk_sb = wts.tile([P, d_chunks, JG], BF16, tag="wk")
            wv_sb = wts.tile([P, d_chunks, JG], BF16, tag="wv")
            wq_sb = wts.tile([P, d_chunks, JG], BF16, tag="wq")
            wo_sb = wts.tile([P, j_chunks, D], BF16, tag="wo")
            bq_sb = consts.tile([P, j_chunks], F32, tag="bq")
            bk_sb = consts.tile([P, j_chunks], F32, tag="bk")

            kT = acts.tile([P, j_chunks, seq], BF16, tag="kT")
            qT = acts.tile([P, j_chunks, seq], BF16, tag="qT")
            # V in s-major head blocks [V_h | ones]: vpp[:, si, h*VW:(h+1)*VW]
            vpp = acts.tile([P, s_chunks, HPC * VW], BF16, tag="vpp")
            ones_view = vpp.rearrange("p s (h c) -> p s h c", c=VW)[:, :, :, DH]
            nc.vector.memset(ones_view, 1.0)

            # DMA emission order = transfer order: feed the first attention
            # block as early as possible.
            x2tiles, x1tiles = [], []

            def emit_xt(pool, tiles, xt_dram, sb, tag, half=None):
                if half in (None, 0):
                    t = pool.tile([P, d_chunks, 512], BF16, tag=tag,
                                  name=f"{tag}_{sb}")
                    tiles.append(t)
                t = tiles[sb]
                dsl = (slice(None) if half is None else
                       slice(half * 4, (half + 1) * 4))
                nc.sync.dma_start(
                    t[:, dsl],
                    xt_dram.rearrange("(o p) s -> p o s", p=P)[
                        :, dsl, sb * 512:(sb + 1) * 512])

            wkr = wk.rearrange("(o p) j -> p o j", p=P)
            wqr = wq.rearrange("(o p) j -> p o j", p=P)
            nc.sync.dma_start(wk_sb[:, :, 0:P], wkr[:, :, 0:P])
            nc.sync.dma_start(bk_sb[:], bkr[:])
            emit_xt(xt2_pool, x2tiles, x2t, 0, "x2T", 0)
            nc.sync.dma_start(wq_sb[:, :, 0:P], wqr[:, :, 0:P])
            emit_xt(xt2_pool, x2tiles, x2t, 0, "x2T", 1)
            nc.sync.dma_start(bq_sb[:], bqr[:])
            emit_xt(xt1_pool, x1tiles, x1t, 0, "x1T", 0)
            emit_xt(xt1_pool, x1tiles, x1t, 0, "x1T", 1)
            emit_xt(xt1_pool, x1tiles, x1t, 1, "x1T", 0)
            emit_xt(xt1_pool, x1tiles, x1t, 1, "x1T", 1)
            nc.sync.dma_start(wv_sb[:], wv.rearrange("(o p) j -> p o j", p=P))
            emit_xt(xt2_pool, x2tiles, x2t, 1, "x2T")
            emit_xt(xt2_pool, x2tiles, x2t, 2, "x2T")
            emit_xt(xt2_pool, x2tiles, x2t, 3, "x2T")
            emit_xt(xt1_pool, x1tiles, x1t, 2, "x1T")
            emit_xt(xt1_pool, x1tiles, x1t, 3, "x1T")
            nc.sync.dma_start(wk_sb[:, :, P:JG], wkr[:, :, P:JG])
            nc.sync.dma_start(wq_sb[:, :, P:JG], wqr[:, :, P:JG])
            nc.sync.dma_start(wo_sb[:], wo.rearrange("(o p) n -> p o n", p=P))

            pk_live = {}

            def project_jc(xT_s, w_sb, sb, out, bias, pool, jc, dcs=None):
                # one jc half of out[j, sb-slab] = w.T @ xT_s + bias;
                # dcs splits the d contraction into separately emitted
                # pieces (the PSUM tile stays live in pk_live between them)
                ssl = slice(sb * 512, (sb + 1) * 512)
                key = (w_sb.name, sb, jc)
                if dcs is None:
                    dcs = range(d_chunks)
                if key in pk_live:
                    pk = pk_live.pop(key)
                else:
                    pk = pool.tile([P, 512], F32, tag=pool.name[5:],
                                   name=f"pk_{w_sb.name}_{sb}_{jc}")
                for dc in dcs:
                    nc.tensor.matmul(
                        pk[:],
                        w_sb[:, dc, jc * P:(jc + 1) * P],
                        xT_s[:, dc, :],
                        start=(dc == 0), stop=(dc == d_chunks - 1))
                if dcs[-1] == d_chunks - 1:
                    nc.vector.tensor_scalar_add(
                        out[:, jc, ssl], pk[:], bias[:, jc:jc + 1])
                else:
                    pk_live[key] = pk

            def project_v(xT_s, sb):
                # V[s-slab, j] = x2_slab @ Wv into the vpp head blocks
                for q in range(4):
                    si = sb * 4 + q
                    pv = psum_mm.tile([P, JG], F32, tag="mm",
                                      name=f"pvv_{sb}_{q}")
                    for dc in range(d_chunks):
                        nc.tensor.matmul(
                            pv[:],
                            xT_s[:, dc, q * P:(q + 1) * P],
                            wv_sb[:, dc, :],
                            start=(dc == 0), stop=(dc == d_chunks - 1))
                    vv = vpp[:, si].rearrange("p (h c) -> p h c", c=VW)
                    nc.vector.tensor_copy(
                        vv[:, :, 0:DH],
                        pv[:].rearrange("p (h c) -> p h c", c=DH))

            ybounce = dram.tile([seq, D], BF16, tag="yin")
            cts = {}
            pvs = {}
            # PV work is deferred behind the exp stream: entries are stamped
            # with the front (unit) index at emission and popped strictly
            # head-sequentially (the two PV banks serialize heads anyway)
            pend_q = {}
            head_order = []
            front_i = 0

            def emit_pv(sc, h, kc, et):
              with nc.named_scope("attn"):
                first, fin = kc == 0, kc == s_chunks - 1
                if first:
                    pvs[(sc, h)] = [
                        psum_pv.tile([P, 4, VW], F32, tag=f"pv{g}",
                                     name=f"pvb_{sc}_{h}_{g}")
                        for g in range(2)]
                for qc in range(QC):
                    # start/stop bracket the whole PSUM bank (2KB zero
                    # region): only the first sub-region starts it and only
                    # the last one stops it
                    nc.tensor.matmul(
                        pvs[(sc, h)][qc // 4][:, qc % 4, :],
                        et[:, qc * P:(qc + 1) * P],
                        vpp[:, kc, h * VW:(h + 1) * VW],
                        start=first and qc % 4 == 0,
                        stop=fin and qc % 4 == 3)
                if fin:
                    emit_attn_tail(sc, h)

            def pop_pend(lag):
                # emit deferred PV work that the exp stream has run at least
                # `lag` units ahead of, oldest head first
                while head_order:
                    hk = head_order[0]
                    q = pend_q[hk]
                    if not q["items"]:
                        if q["popped"] == s_chunks:
                            head_order.pop(0)
                            continue
                        break
                    idx, kc, et = q["items"][0]
                    if front_i - idx < lag:
                        break
                    q["items"].pop(0)
                    q["popped"] += 1
                    emit_pv(hk[0], hk[1], kc, et)

            def emit_attn_unit(sc, h, kc, lag=3, halves=False):
              nonlocal front_i
              with nc.named_scope("attn"):
                jc, po = h // 2, (h % 2) * DH
                if (h, kc) == (0, 0):
                    cts[sc] = ctp.tile([P, j_chunks, QC * P], BF16, tag="ct",
                                       name=f"ct_{sc}")
                if kc == 0:
                    pend_q[(sc, h)] = {"items": [], "popped": 0}
                    head_order.append((sc, h))
                ps = psum_s.tile([P, 1024], F32, tag="s",
                                 name=f"ps_{sc}_{h}_{kc}")
                et = epool.tile([P, 1024], BF16, tag="e",
                                name=f"et_{sc}_{h}_{kc}")
                for half in range(2):
                    fsl = slice(half * 512, (half + 1) * 512)
                    hsl = slice(sc * 1024 + half * 512,
                                sc * 1024 + (half + 1) * 512)
                    nc.tensor.matmul(
                        ps[:, fsl],
                        kT[po:po + DH, jc, kc * P:(kc + 1) * P],
                        qT[po:po + DH, jc, hsl],
                        start=True, stop=True)
                    if halves:
                        # pipeline-fill: exp each half as soon as its score
                        # lands (the first q slab arrives before the second)
                        nc.scalar.activation(et[:, fsl], ps[:, fsl], EXP,
                                             scale=0.125)
                if not halves:
                    nc.scalar.activation(et[:], ps[:], EXP, scale=0.125)
                pend_q[(sc, h)]["items"].append((front_i, kc, et))
                front_i += 1
                pop_pend(lag)

            def emit_attn_tail(sc, h):
                # normalize ctx during eviction, then transpose to j-major.
                # For the very last head ACT is already done exp'ing, so
                # half the eviction chain runs there to shorten the tail.
                last = (sc, h) == (sk_chunks - 1, HPC - 1)
                jc, po = h // 2, (h % 2) * DH
                pvb = pvs.pop((sc, h))
                ct = cts[sc]
                with nc.named_scope("norm"):
                    rc = rpool.tile([P, 2, 4], F32, tag="rc",
                                    name=f"rc_{sc}_{h}")
                    for g in range(2):
                        nc.vector.reciprocal(rc[:, g], pvb[g][:, :, DH])
                    cn = cnp.tile([P, QC, DH], BF16, tag="cn",
                                  name=f"cn_{sc}_{h}")
                    for qc in range(QC):
                        if last and qc % 2 == 0:
                            nc.scalar.mul(
                                cn[:, qc, :], pvb[qc // 4][:, qc % 4, 0:DH],
                                rc[:, qc // 4, qc % 4:qc % 4 + 1])
                        else:
                            nc.vector.tensor_scalar_mul(
                                cn[:, qc, :], pvb[qc // 4][:, qc % 4, 0:DH],
                                rc[:, qc // 4, qc % 4:qc % 4 + 1])
                with nc.named_scope("ctT"):
                    for g in range(2):
                        pt = psum_mm.tile([P, 4, P], BF16, tag="mm",
                                          name=f"pt_{sc}_{h}_{g}")
                        for i in range(4):
                            # one accumulation group per PSUM bank: first
                            # transpose starts it, last one stops it
                            nc.tensor.matmul(
                                pt[0:DH, i, :], cn[:, g * 4 + i, :],
                                ident[:], is_transpose=True,
                                start=(i == 0), stop=(i == 3))
                        out = ct[po:po + DH, jc,
                                 g * 512:(g + 1) * 512].rearrange(
                                     "p (a b) -> p a b", a=4)
                        if last and g == 0:
                            nc.scalar.copy(out, pt[0:DH])
                        else:
                            nc.vector.tensor_copy(out, pt[0:DH])

            def emit_qproj_late(sb):
                # Q slabs 2-3 woven into the sc0 attention stream (mm banks)
                project_jmajor(x1tiles[sb], wq_sb, sb, qT, bq_sb, psum_mm)

            yp_sb = ypart.tile([P, QC, D], BF16, tag="yp")

            def emit_oproj(sc, s8_list, rows, mode, pool):
                # out-projection pass over 128-row chunks of Y contracting
                # the `rows` slice of ct/wo. mode: "full" (single pass),
                # "park" (first pass -> yp_sb), "acc" (add into yp_sb),
                # "fin" (last pass: re-add yp_sb in PSUM via an identity
                # matmul, evict split over ACT/DVE -- this is the kernel
                # tail, keep it short).
                ct = cts[sc]
                r0, r1 = rows
                jcs = (range(j_chunks) if (r0, r1) == (0, JG) else
                       [r0 // P])
                p0, p1 = r0 % P, (r1 - 1) % P + 1
                for s8 in s8_list:
                  with nc.named_scope("oproj"):
                    if mode in ("full", "fin"):
                        yt = ysb.tile([P, D], BF16, tag="y",
                                      name=f"yt_{sc}_{s8}")
                    for nck in range(2):
                        py = pool.tile([P, 512], F32, tag=pool.name[5:],
                                       name=f"py_{sc}_{s8}_{nck}_{r0}")
                        on_act = mode == "fin" and (s8 + nck) % 2 == 0
                        for i, jc in enumerate(jcs):
                            nc.tensor.matmul(
                                py[:],
                                ct[p0:p1, jc, s8 * P:(s8 + 1) * P],
                                wo_sb[p0:p1, jc, nck * 512:(nck + 1) * 512],
                                start=(i == 0),
                                stop=(i == len(jcs) - 1 and not on_act))
                        osl = slice(nck * 512, (nck + 1) * 512)
                        if mode == "park":
                            nc.vector.tensor_copy(yp_sb[:, s8, osl], py[:])
                        elif mode == "acc":
                            nc.vector.tensor_tensor(
                                yp_sb[:, s8, osl], py[:], yp_sb[:, s8, osl],
                                ADD)
                        elif on_act:
                            # ACT can't add two tensors: fold the parked
                            # partial back into PSUM via an identity matmul,
                            # then a plain scalar-engine copy evicts it
                            nc.tensor.matmul(
                                py[:], ident[:],
                                yp_sb[:, s8, osl],
                                start=False, stop=True)
                            nc.scalar.copy(yt[:, osl], py[:])
                        elif mode == "fin":
                            nc.vector.tensor_tensor(
                                yt[:, osl], py[:], yp_sb[:, s8, osl], ADD)
                        else:
                            nc.vector.tensor_copy(yt[:, osl], py[:])
                    if mode in ("full", "fin"):
                        nc.sync.dma_start(
                            ybounce[(sc * QC + s8) * P:
                                    (sc * QC + s8 + 1) * P, :],
                            yt[:])

            def qlate_piece(sb, jc):
                # one jc-half of a late Q projection slab (mm banks)
                def f():
                    ssl = slice(sb * 512, (sb + 1) * 512)
                    pk = psum_mm.tile([P, 512], F32, tag="mm",
                                      name=f"pkl_{sb}_{jc}")
                    for dc in range(d_chunks):
                        nc.tensor.matmul(
                            pk[:],
                            wq_sb[:, dc, jc * P:(jc + 1) * P],
                            x1tiles[sb][:, dc, :],
                            start=(dc == 0), stop=(dc == d_chunks - 1))
                    nc.vector.tensor_scalar_add(
                        qT[:, jc, ssl], pk[:], bq_sb[:, jc:jc + 1])
                return f

            def oproj_piece(sc, s8, rows, mode, pool):
                return lambda: emit_oproj(sc, [s8], rows, mode, pool)

            def kjc1_piece(sb):
                return lambda: project_jc(x2tiles[sb], wk_sb, sb, kT, bk_sb,
                                          psum_mm, 1)

            def run_units(sc, h, side, lag=3):
                # side: kc -> closure emitting independent PE work, woven
                # between units to keep PE busy while ACT paces the stream
                for kc in range(s_chunks):
                    emit_attn_unit(sc, h, kc, lag)
                    f = side.get(kc)
                    if f is not None:
                        f()

            def run_units2(ha, hb, side, lag=3):
                # two heads' units interleaved (PV pops stay
                # head-sequential); side pieces woven after the pairs
                for kc in range(s_chunks):
                    emit_attn_unit(ha[0], ha[1], kc, lag)
                    emit_attn_unit(hb[0], hb[1], kc, lag)
                    f = side.get(kc)
                    if f is not None:
                        f()

            # ---- phase 1: projections woven into the first two heads of
            # attention block sc0. Only the jc0 halves of K/Q are needed by
            # heads 0-1, so jc1 is deferred to the head-2/3 stream. Head 0
            # and head 1 exp units interleave (PV pops are head-sequential
            # behind the exp stream), keeping the scalar engine fed while
            # PE grinds through projections. ----
            LAG1 = 12
            with nc.named_scope("proj0"):
                project_jc(x2tiles[0], wk_sb, 0, kT, bk_sb, psum_mm, 0,
                           [0, 1, 2, 3])
                project_jc(x1tiles[0], wq_sb, 0, qT, bq_sb, psum_mm, 0,
                           [0, 1, 2, 3])
                project_jc(x2tiles[0], wk_sb, 0, kT, bk_sb, psum_mm, 0,
                           [4, 5, 6, 7])
                project_jc(x1tiles[0], wq_sb, 0, qT, bq_sb, psum_mm, 0,
                           [4, 5, 6, 7])
                project_jc(x1tiles[1], wq_sb, 1, qT, bq_sb, psum_mm, 0)
            for sb in range(1, sb_chunks):
                for kc in range((sb - 1) * 4, sb * 4):
                    emit_attn_unit(0, 0, kc, LAG1, halves=(kc < 3))
                    if sb == 1 and kc == 2:
                        # V0 is first needed by the PV pops ~LAG1 units in;
                        # keeping it off the critical path lets the first
                        # scores flow straight into the exp pipeline
                        with nc.named_scope("proj0v"):
                            project_v(x2tiles[0], 0)
                for kc in range((sb - 1) * 4, sb * 4):
                    emit_attn_unit(0, 1, kc, LAG1)
                with nc.named_scope(f"proj{sb}"):
                    project_jc(x2tiles[sb], wk_sb, sb, kT, bk_sb, psum_mm, 0)
                    project_v(x2tiles[sb], sb)
            # jc1 projections for heads 2-3 are woven into the last
            # phase-1 units (slabs 1-3 between head-2 units further down)
            for kc in range(12, s_chunks):
                emit_attn_unit(0, 0, kc, LAG1)
                if kc == 12:
                    project_jc(x2tiles[0], wk_sb, 0, kT, bk_sb, psum_mm, 1)
                elif kc == 14:
                    project_jc(x1tiles[0], wq_sb, 0, qT, bq_sb, psum_mm, 1)
            for kc in range(12, s_chunks):
                emit_attn_unit(0, 1, kc, LAG1)
                if kc == 12:
                    project_jc(x1tiles[1], wq_sb, 1, qT, bq_sb, psum_mm, 1)
            run_units2((0, 2), (0, 3),
                       {1: kjc1_piece(1), 3: qlate_piece(2, 0),
                        5: kjc1_piece(2), 7: qlate_piece(2, 1),
                        9: kjc1_piece(3), 11: qlate_piece(3, 0),
                        13: qlate_piece(3, 1)}, 7)

            # ---- phase 2: attention sc1; oproj(sc0) + partial oproj(sc1)
            # passes woven in so only the last per-head pass lands in the
            # tail. ct0 completes while (1,0) is in flight; ct1 jc0 during
            # (1,2); ct1 head 2 during (1,3). ----
            o0 = [oproj_piece(0, s8, (0, JG), "full", psum_mm)
                  for s8 in range(QC)]
            pk1 = [oproj_piece(1, s8, (0, P), "park", psum_mm)
                   for s8 in range(QC)]
            ac1 = [oproj_piece(1, s8, (P, P + DH), "acc", psum_mm)
                   for s8 in range(QC)]
            run_units(1, 0, {7: o0[0], 11: o0[1], 15: o0[2]}, 5)
            run_units(1, 1, {3: o0[3], 7: o0[4], 11: o0[5], 15: o0[6]}, 4)
            run_units(1, 2, {1: o0[7], 5: pk1[0], 7: pk1[1], 9: pk1[2],
                             11: pk1[3], 13: pk1[4], 15: pk1[5]}, 3)
            run_units(1, 3, {1: pk1[6], 3: pk1[7], 5: ac1[0], 7: ac1[1],
                             9: ac1[2], 11: ac1[3], 13: ac1[4], 15: ac1[5]},
                      3)
            ac1[6]()
            ac1[7]()
            while head_order:
                pop_pend(0)
            for s8 in range(QC):
                emit_oproj(1, [s8], (P + DH, JG), "fin",
                           psum_s if s8 % 2 else psum_mm)

            # ---- sum partials across the 4 cores of this batch ----
            if with_collective:
                half = seq // 2                 # 1024 rows per collective
                qr = seq // GROUPS // 2         # 256 rows per rank per half
                for ci in range(2):
                    ysc = dram.tile([qr, D], BF16, tag="yout",
                                    name=f"ysc_{ci}")
                    nc.gpsimd.collective_compute(
                        "ReduceScatter",
                        mybir.AluOpType.add,
                        replica_groups=[[0, 1, 2, 3], [4, 5, 6, 7]],
                        ins=[ybounce[ci * half:(ci + 1) * half, :].opt()],
                        outs=[ysc[:].opt()],
                    )
                    nc.sync.dma_start(y_out[ci * qr:(ci + 1) * qr, :], ysc[:])
            else:
                nc.sync.dma_start(y_out[:], ybounce[:seq // GROUPS, :])

    nc.compile()
    return nc


def _get_program(seq=SEQ):
    if seq not in _cached:
        _cached[seq] = _build_program(seq)
    return _cached[seq]


def make_in_maps(x1, x2, Wq, bq, Wk, bk, Wv, bv, Wo, bo):
    """Per-core input dicts for the SPMD program."""
    bf16 = ml_dtypes.bfloat16
    in_maps = []
    for c in range(N_CORES):
        b, g = c // GROUPS, c % GROUPS
        js = slice(g * JG, (g + 1) * JG)
        in_maps.append({
            "x1t": np.ascontiguousarray(x1[b].T.astype(bf16)),
            "x2t": np.ascontiguousarray(x2[b].T.astype(bf16)),
            "wq": np.ascontiguousarray(Wq[:, js].astype(bf16)),
            "wk": np.ascontiguousarray(Wk[:, js].astype(bf16)),
            "wv": np.ascontiguousarray(Wv[:, js].astype(bf16)),
            "wo": np.ascontiguousarray(Wo[js, :].astype(bf16)),
            "bqr": np.ascontiguousarray(bq[js].reshape(2, P).T),
            "bkr": np.ascontiguousarray(bk[js].reshape(2, P).T),
        })
    return in_maps


def assemble(results, Wv_bias_fix):
    """results: list of per-core {'y_out': [seq//GROUPS, D]}.

    y_out rows [0:q) = rank's quarter of input rows [0:seq/2);
    rows [q:2q) = rank's quarter of input rows [seq/2:seq)."""
    seq = results[0]["y_out"].shape[0] * GROUPS
    q = seq // GROUPS // 2
    Y = np.empty((B, seq, D), np.float32)
    for c in range(N_CORES):
        b, rr = c // GROUPS, c % GROUPS
        yo = np.asarray(results[c]["y_out"]).astype(np.float32)
        Y[b, rr * q:(rr + 1) * q, :] = yo[:q]
        Y[b, seq // 2 + rr * q:seq // 2 + (rr + 1) * q, :] = yo[q:]
    Y += Wv_bias_fix
    return Y


def kernel(x1, x2, Wq, bq, Wk, bk, Wv, bv, Wo, bo):
    from concourse.bass_utils import run_bass_kernel_spmd

    x1 = np.asarray(x1, np.float32)
    x2 = np.asarray(x2, np.float32)
    Wq, bq = np.asarray(Wq, np.float32), np.asarray(bq, np.float32)
    Wk, bk = np.asarray(Wk, np.float32), np.asarray(bk, np.float32)
    Wv, bv = np.asarray(Wv, np.float32), np.asarray(bv, np.float32)
    Wo, bo = np.asarray(Wo, np.float32), np.asarray(bo, np.float32)

    nc = _get_program(SEQ)
    in_maps = make_in_maps(x1, x2, Wq, bq, Wk, bk, Wv, bv, Wo, bo)
    res = run_bass_kernel_spmd(nc, in_maps, core_ids=list(range(N_CORES)))
    fix = (bv @ Wo + bo).astype(np.float32)
    return assemble(res.results, fix)
